# revision 4
# baseline (speedup 1.0000x reference)
"""DelayGIN message-passing kernel for Trainium2 (8 NeuronCores).

Strategy (edge partitioning by destination node):
  - Nodes are sharded contiguously across the 8 cores (6250 each). Every core
    owns exactly the edges whose dst lands in its shard, so segment-sum is
    fully local (no cross-core reduction of accumulators).
  - Per (relation, dst-window of 128 nodes) the edges are gathered from a
    replicated bf16 copy of the source features via SWDGE dma_gather, and
    scatter-added with a PE matmul against a DVE-built one-hot matrix:
        aggT[D, W] += msg_tile[128e, D].T @ onehot[128e, W]
  - mlp_k / mlp_s run as [D,D] x [D, nodes] matmuls on the transposed shard,
    relu+bias on ACT, accumulated on DVE. Residual keeps the fp32 state xT
    resident in SBUF. Per layer the new x shard is PE-transposed back to
    node-major, cast to bf16, and AllGathered so every core can gather from
    the full x_{t+1}. The final layer is written out in fp32, and the host
    concatenates the 8 shard outputs.
  - Layer t uses relations k=1..t+1 with source history x_{t-(k-1)}; relation
    edge lists/one-hot metadata are precomputed host-side (they do not depend
    on feature values) and shipped as int16 inputs. All 8 cores run one SPMD
    program; per-(rel,window,group) tile counts are padded to the max across
    cores so the instruction schedule is shared.
"""

import os
import sys

for _p in ("/opt/trn_rl_repo", "/root/.axon_site/_ro/trn_rl_repo"):
    if os.path.isdir(_p) and _p not in sys.path:
        sys.path.insert(0, _p)

import numpy as np
import ml_dtypes

import concourse.bass as bass
import concourse.bacc as bacc
import concourse.mybir as mybir
import concourse.tile as tile
from concourse.bass_utils import run_bass_kernel_spmd
from concourse.masks import make_identity

P = 128
N = 50000
E = 1500000
D = 128
L = 3
NCORES = 8
NS = N // NCORES          # 6250 nodes per core
W = 128                   # dst window (psum free dim)
NW = (NS + W - 1) // W    # 49 windows; last has 106 nodes
SPLIT = 32768             # src-index split so gather indices fit in int16
CW = 7                    # windows per gather chunk (49 = 7*7)
NCHUNK = NW // CW         # 7 chunks
MLPC = 512                # mlp node-chunk (psum free dim)
OHB = 8                   # one-hot tiles built per DVE instruction

f32 = mybir.dt.float32
bf16 = mybir.dt.bfloat16
i16 = mybir.dt.int16

# (t, r) -> which x history the gather reads: x_{t-r}; r=k-1
PASSES = [(t, r) for t in range(L) for r in range(t, -1, -1)]


def _wrap_idx(idx):
    # [n] -> [128, n//16] int16; element i at [i%16, i//16], replicated x8.
    arr = idx.astype(np.int16).reshape(-1, 16).T
    return np.tile(arr, (8, 1))


def _prepare(edge_index, edge_attr):
    """Host-side edge partitioning. Returns (schedule, per-core meta arrays)."""
    src = np.asarray(edge_index[0], dtype=np.int64)
    dst = np.asarray(edge_index[1], dtype=np.int64)
    attr = np.asarray(edge_attr, dtype=np.int64)

    # counts[c, r, g, w]
    counts = np.zeros((NCORES, L, 2, NW), dtype=np.int64)
    per_cr = {}
    shard = dst // NS
    for c in range(NCORES):
        for r in range(L):
            sel = (attr == r) & (shard == c)
            s = src[sel]
            dl = dst[sel] - c * NS
            g = (s >= SPLIT).astype(np.int64)
            w = dl // W
            for gi in (0, 1):
                m = g == gi
                counts[c, r, gi] = np.bincount(w[m], minlength=NW)
            per_cr[(c, r)] = (s, dl, g, w)

    # shared schedule: tiles per (r, w, g) = max over cores
    ntile = np.ceil(counts.max(axis=0) / P).astype(np.int64)  # [L, 2, NW]
    ntile = np.maximum(ntile, 1)  # keep >=1 so windows always have a group
    # per-(r,g) stream tile offsets by window
    tbase = np.zeros((L, 2, NW + 1), dtype=np.int64)
    tbase[:, :, 1:] = np.cumsum(ntile, axis=2)

    # column layout of the per-relation meta tensor:
    #   idx_g0 (nt0*8) | idx_g1 (nt1*8) | dl_g0 (nt0) | dl_g1 (nt1)
    nt_tot = tbase[:, :, -1]  # [L, 2]
    meta_cols = [int(nt_tot[r, 0] * 8 + nt_tot[r, 1] * 8 + nt_tot[r, 0] + nt_tot[r, 1])
                 for r in range(L)]

    metas = {}
    for c in range(NCORES):
        for r in range(L):
            s, dl, g, w = per_cr[(c, r)]
            parts = []
            dls = []
            for gi in (0, 1):
                m = g == gi
                sw = w[m]
                order = np.argsort(sw, kind="stable")
                sw = sw[order]
                sidx = (s[m] - gi * SPLIT)[order]
                sdl = (dl[m] % W)[order]
                total = int(nt_tot[r, gi]) * P
                # position = stream offset of window + rank within window
                cnt = counts[c, r, gi]
                within = np.arange(sw.size) - np.repeat(
                    np.concatenate([[0], np.cumsum(cnt)[:-1]]), cnt)
                pos = tbase[r, gi][sw] * P + within
                idx_pad = np.zeros(total, dtype=np.int16)
                dl_pad = np.full(total, -1, dtype=np.int16)
                idx_pad[pos] = sidx.astype(np.int16)
                dl_pad[pos] = sdl.astype(np.int16)
                parts.append(_wrap_idx(idx_pad))
                dls.append(dl_pad.reshape(-1, P).T.copy())
            metas[(c, r)] = np.concatenate(parts + dls, axis=1).astype(np.int16)

    sched = dict(ntile=ntile, tbase=tbase, nt_tot=nt_tot, meta_cols=meta_cols)
    return sched, metas


def _build(sched):
    ntile, tbase, nt_tot = sched["ntile"], sched["tbase"], sched["nt_tot"]
    meta_cols_max = max(sched["meta_cols"])

    # max tiles per (chunk, group) for gather/onehot buffer sizing
    def chunk_tiles(r, k, g):
        return int(tbase[r, g, min((k + 1) * CW, NW)] - tbase[r, g, k * CW])

    TA = max(chunk_tiles(r, k, 0) for r in range(L) for k in range(NCHUNK))
    TB = max(chunk_tiles(r, k, 1) for r in range(L) for k in range(NCHUNK))

    nc = bacc.Bacc("TRN2", num_devices=NCORES)

    d_x0bf = nc.dram_tensor("x0bf", [N, D], bf16, kind="ExternalInput")
    d_xT0 = nc.dram_tensor("xT0", [P, NS], f32, kind="ExternalInput")
    d_wts = nc.dram_tensor("wts", [P, 12 * D], f32, kind="ExternalInput")
    d_bias = nc.dram_tensor("bias", [P, 12], f32, kind="ExternalInput")
    d_meta = [nc.dram_tensor(f"meta{r}", [P, sched["meta_cols"][r]], i16,
                             kind="ExternalInput") for r in range(L)]
    d_out = nc.dram_tensor("out", [NS, D], f32, kind="ExternalOutput")

    ag_in = [nc.dram_tensor(f"ag_in{t}", [NS, D], bf16, kind="Internal")
             for t in range(L - 1)]
    x_hist = [d_x0bf]
    for t in range(L - 1):
        x_hist.append(nc.dram_tensor(f"x{t + 1}bf", [N, D], bf16,
                                     kind="Internal", addr_space="Shared"))

    # weight slot order: (t, r) -> t*3+r ; self t -> 9+t
    def wslot(t, r):
        return t * 3 + r

    def sslot(t):
        return 9 + t

    # mlp node chunks
    mlp_chunks = []
    c0 = 0
    while c0 < NS:
        cw = min(MLPC, NS - c0)
        mlp_chunks.append((c0, cw))
        c0 += cw

    with tile.TileContext(nc) as tc:
        with (
            tc.tile_pool(name="persist", bufs=1) as pp,
            tc.tile_pool(name="meta", bufs=2) as pmeta,
            tc.tile_pool(name="mbuf", bufs=2) as pm,
            tc.tile_pool(name="ohbuf", bufs=2) as poh,
            tc.tile_pool(name="aggc", bufs=3) as paggc,
            tc.tile_pool(name="stage", bufs=3) as pstage,
            tc.tile_pool(name="xout", bufs=2) as pxout,
            tc.tile_pool(name="pagg", bufs=4, space="PSUM") as ps_agg,
            tc.tile_pool(name="pmlp", bufs=2, space="PSUM") as ps_mlp,
            tc.tile_pool(name="ptr", bufs=2, space="PSUM") as ps_tr,
        ):
            t_iota = pp.tile([P, P], i16, tag="iota")
            t_ident = pp.tile([P, P], f32, tag="ident")
            t_xT = pp.tile([P, NS], f32, tag="xT")
            t_acc = pp.tile([P, NS], f32, tag="acc")
            t_wall = pp.tile([P, 12 * D], f32, tag="wall")
            t_ball = pp.tile([P, 12], f32, tag="ball")

            nc.gpsimd.iota(t_iota[:], pattern=[[1, P]], base=0,
                           channel_multiplier=0)
            make_identity(nc, t_ident[:])
            nc.sync.dma_start(t_xT[:], d_xT0[:])
            nc.sync.dma_start(t_wall[:], d_wts[:])
            nc.sync.dma_start(t_ball[:], d_bias[:])

            def emit_mlp_term(t, slot, rhs_tile_of_chunk, first):
                """out_acc (+)= relu(W.T-applied rhs + b) over node chunks."""
                for (c0_, cw) in mlp_chunks:
                    pmlp = ps_mlp.tile([P, MLPC], f32, tag="pmlp")
                    nc.tensor.matmul(
                        pmlp[:, :cw],
                        lhsT=t_wall[:, slot * D:(slot + 1) * D],
                        rhs=rhs_tile_of_chunk(c0_, cw),
                        start=True, stop=True,
                    )
                    if first:
                        nc.scalar.activation(
                            t_acc[:, c0_:c0_ + cw], pmlp[:, :cw],
                            mybir.ActivationFunctionType.Relu,
                            bias=t_ball[:, slot:slot + 1])
                    else:
                        tmp = pstage.tile([P, MLPC], f32, tag="mlptmp")
                        nc.scalar.activation(
                            tmp[:, :cw], pmlp[:, :cw],
                            mybir.ActivationFunctionType.Relu,
                            bias=t_ball[:, slot:slot + 1])
                        nc.vector.tensor_add(
                            t_acc[:, c0_:c0_ + cw],
                            t_acc[:, c0_:c0_ + cw], tmp[:, :cw])

            for t in range(L):
                # self term first (only needs resident xT)
                emit_mlp_term(
                    t, sslot(t),
                    lambda c0_, cw: t_xT[:, c0_:c0_ + cw],
                    first=True)

                # relations, oldest source first (overlaps the AllGather)
                for r in range(t, -1, -1):
                    x_src = x_hist[t - r]
                    t_meta = pmeta.tile([P, meta_cols_max], i16, tag="meta")
                    nc.sync.dma_start(t_meta[:, :sched["meta_cols"][r]],
                                      d_meta[r][:])
                    icol = [0, int(nt_tot[r, 0]) * 8]
                    dcol = [int(nt_tot[r, 0] + nt_tot[r, 1]) * 8,
                            int((nt_tot[r, 0] + nt_tot[r, 1]) * 8 + nt_tot[r, 0])]

                    agg_tiles = {}  # mlp-chunk idx -> (tile, emitted windows)
                    for k in range(NCHUNK):
                        w_lo, w_hi = k * CW, min((k + 1) * CW, NW)
                        mb = [None, None]
                        ohb = [None, None]
                        cstart = [int(tbase[r, g, w_lo]) for g in (0, 1)]
                        for g in (0, 1):
                            ntk = chunk_tiles(r, k, g)
                            if ntk == 0:
                                continue
                            cap = TA if g == 0 else TB
                            mb[g] = pm.tile([P, cap, P], bf16,
                                            tag=f"m{g}", name=f"mb{g}")
                            nidx = ntk * P
                            src_view = (x_src[:SPLIT, :] if g == 0
                                        else x_src[SPLIT:N, :])
                            nc.gpsimd.dma_gather(
                                mb[g][:, :ntk, :], src_view,
                                t_meta[:, icol[g] + cstart[g] * 8:
                                       icol[g] + (cstart[g] + ntk) * 8],
                                nidx, nidx, D, single_packet=False)
                            ohb[g] = poh.tile([P, cap, P], bf16,
                                              tag=f"oh{g}", name=f"ohb{g}")
                            for j0 in range(0, ntk, OHB):
                                bs = min(OHB, ntk - j0)
                                a = t_iota[:].rearrange(
                                    "p (t w) -> p t w", t=1)
                                b = t_meta[:, dcol[g] + cstart[g] + j0:
                                           dcol[g] + cstart[g] + j0 + bs
                                           ].rearrange("p (t w) -> p t w", w=1)
                                a2, b2 = bass.broadcast_tensor_aps(a, b)
                                nc.vector.tensor_tensor(
                                    out=ohb[g][:, j0:j0 + bs, :],
                                    in0=a2, in1=b2,
                                    op=mybir.AluOpType.is_equal)

                        for w in range(w_lo, w_hi):
                            wn = min(W, NS - w * W)
                            mc = (w * W) // MLPC
                            if mc not in agg_tiles:
                                agg_tiles[mc] = paggc.tile(
                                    [P, MLPC], f32, tag="aggT", name="aggc")
                            at = agg_tiles[mc]
                            # window tile list
                            ops = []
                            for g in (0, 1):
                                lo = int(tbase[r, g, w]) - cstart[g]
                                hi = int(tbase[r, g, w + 1]) - cstart[g]
                                for j in range(lo, hi):
                                    ops.append((mb[g], ohb[g], j))
                            pagg = ps_agg.tile([P, W], f32, tag="pagg")
                            for oi, (mbt, oht, j) in enumerate(ops):
                                nc.tensor.matmul(
                                    pagg[:],
                                    lhsT=mbt[:, j, :],
                                    rhs=oht[:, j, :],
                                    start=(oi == 0),
                                    stop=(oi == len(ops) - 1))
                            off = (w * W) % MLPC
                            nc.scalar.copy(at[:, off:off + wn],
                                           pagg[:, :wn])
                            # mlp chunk complete?
                            chunk_end = (w * W + wn)
                            if chunk_end % MLPC == 0 or chunk_end == NS:
                                c0_ = mc * MLPC
                                cw = chunk_end - c0_
                                at_ref = agg_tiles.pop(mc)
                                pmlp = ps_mlp.tile([P, MLPC], f32, tag="pmlp")
                                slot = wslot(t, r)
                                nc.tensor.matmul(
                                    pmlp[:, :cw],
                                    lhsT=t_wall[:, slot * D:(slot + 1) * D],
                                    rhs=at_ref[:, :cw],
                                    start=True, stop=True)
                                tmp = pstage.tile([P, MLPC], f32, tag="mlptmp")
                                nc.scalar.activation(
                                    tmp[:, :cw], pmlp[:, :cw],
                                    mybir.ActivationFunctionType.Relu,
                                    bias=t_ball[:, slot:slot + 1])
                                nc.vector.tensor_add(
                                    t_acc[:, c0_:c0_ + cw],
                                    t_acc[:, c0_:c0_ + cw], tmp[:, :cw])

                # finalize layer: x = x + relu(acc)
                for (c0_, cw) in mlp_chunks:
                    tmp = pstage.tile([P, MLPC], f32, tag="mlptmp")
                    nc.scalar.activation(
                        tmp[:, :cw], t_acc[:, c0_:c0_ + cw],
                        mybir.ActivationFunctionType.Relu)
                    nc.vector.tensor_add(
                        t_xT[:, c0_:c0_ + cw],
                        t_xT[:, c0_:c0_ + cw], tmp[:, :cw])

                # write node-major copy (bf16 for t<2 via AllGather; f32 out at t=2)
                WB = 8  # windows per writeback batch
                for b0 in range(0, NW, WB):
                    bw = min(WB, NW - b0)
                    full_rows = min(NS, (b0 + bw) * W) - b0 * W
                    if t < L - 1:
                        st = pxout.tile([P, WB, D], bf16, tag="xbf")
                    else:
                        st = pxout.tile([P, WB, D], f32, tag="xf32")
                    for bi in range(bw):
                        w = b0 + bi
                        wn = min(W, NS - w * W)
                        ptr = ps_tr.tile([P, P], f32, tag="ptr")
                        nc.tensor.transpose(
                            out=ptr[:wn, :],
                            in_=t_xT[:, w * W:w * W + wn],
                            identity=t_ident[:])
                        nc.vector.tensor_copy(st[:wn, bi, :], ptr[:wn, :])
                    dstt = ag_in[t] if t < L - 1 else d_out
                    if full_rows % P == 0:
                        view = dstt[b0 * W: b0 * W + full_rows, :].rearrange(
                            "(c p) d -> p c d", p=P)
                        nc.sync.dma_start(view, st[:, :bw, :])
                    else:
                        nfull = full_rows // P
                        if nfull:
                            view = dstt[b0 * W: b0 * W + nfull * P, :].rearrange(
                                "(c p) d -> p c d", p=P)
                            nc.sync.dma_start(view, st[:, :nfull, :])
                        rem = full_rows - nfull * P
                        nc.sync.dma_start(
                            dstt[b0 * W + nfull * P: b0 * W + full_rows, :],
                            st[:rem, nfull, :])

                if t < L - 1:
                    nc.gpsimd.collective_compute(
                        "AllGather",
                        mybir.AluOpType.bypass,
                        replica_groups=[list(range(NCORES))],
                        ins=[ag_in[t][:]],
                        outs=[x_hist[t + 1][:]],
                    )

    nc.compile()
    return nc


_CACHE = {}


def kernel(x, Ws_s, bs_s, Ws_k, bs_k, edge_index, edge_attr):
    x = np.asarray(x, dtype=np.float32)
    Ws_s = np.asarray(Ws_s, dtype=np.float32)
    bs_s = np.asarray(bs_s, dtype=np.float32)
    Ws_k = np.asarray(Ws_k, dtype=np.float32)
    bs_k = np.asarray(bs_k, dtype=np.float32)
    edge_index = np.asarray(edge_index)
    edge_attr = np.asarray(edge_attr)

    key = hash((edge_index.tobytes(), edge_attr.tobytes()))
    if key not in _CACHE:
        sched, metas = _prepare(edge_index, edge_attr)
        nc = _build(sched)
        _CACHE[key] = (sched, metas, nc)
    sched, metas, nc = _CACHE[key]

    # pack weights: [128, 12*128]: slots t*3+r -> Ws_k[t, r]; 9+t -> Ws_s[t]
    wall = np.zeros((P, 12 * D), np.float32)
    ball = np.zeros((P, 12), np.float32)
    for t in range(L):
        for r in range(L):
            wall[:, (t * 3 + r) * D:(t * 3 + r + 1) * D] = Ws_k[t, r]
            ball[:, t * 3 + r] = bs_k[t, r]
        wall[:, (9 + t) * D:(10 + t) * D] = Ws_s[t]
        ball[:, 9 + t] = bs_s[t]

    x0bf = x.astype(ml_dtypes.bfloat16)
    in_maps = []
    for c in range(NCORES):
        m = {
            "x0bf": x0bf,
            "xT0": np.ascontiguousarray(x[c * NS:(c + 1) * NS].T),
            "wts": wall,
            "bias": ball,
        }
        for r in range(L):
            m[f"meta{r}"] = metas[(c, r)]
        in_maps.append(m)

    trace = bool(int(os.environ.get("KERNEL_TRACE", "0")))
    res = run_bass_kernel_spmd(nc, in_maps, core_ids=list(range(NCORES)),
                               trace=trace)
    if trace and res.exec_time_ns is not None:
        print(f"HW exec time: {res.exec_time_ns} ns")
        kernel.last_exec_time_ns = res.exec_time_ns
        if res.instructions_and_trace is not None:
            print("trace:", res.instructions_and_trace[1])

    out = np.concatenate([res.results[c]["out"] for c in range(NCORES)], axis=0)
    return out


# revision 5
# speedup vs baseline: 1.1688x; 1.1688x over previous
"""DelayGIN message-passing kernel for Trainium2 (8 NeuronCores).

Strategy (edge partitioning by destination node):
  - Nodes are sharded contiguously across the 8 cores (6250 each). Every core
    owns exactly the edges whose dst lands in its shard, so segment-sum is
    fully local (no cross-core reduction of accumulators).
  - Per (relation, dst-window of 128 nodes) the edges are gathered from a
    replicated bf16 copy of the source features via SWDGE dma_gather, and
    scatter-added with a PE matmul against a DVE-built one-hot matrix:
        aggT[D, W] += msg_tile[128e, D].T @ onehot[128e, W]
  - mlp_k / mlp_s run as [D,D] x [D, nodes] matmuls on the transposed shard,
    relu+bias on ACT, accumulated on DVE. Residual keeps the fp32 state xT
    resident in SBUF. Per layer the new x shard is PE-transposed back to
    node-major, cast to bf16, and AllGathered so every core can gather from
    the full x_{t+1}. The final layer is written out in fp32, and the host
    concatenates the 8 shard outputs.
  - Layer t uses relations k=1..t+1 with source history x_{t-(k-1)}; relation
    edge lists/one-hot metadata are precomputed host-side (they do not depend
    on feature values) and shipped as int16 inputs. All 8 cores run one SPMD
    program; per-(rel,window,group) tile counts are padded to the max across
    cores so the instruction schedule is shared.
"""

import os
import sys

for _p in ("/opt/trn_rl_repo", "/root/.axon_site/_ro/trn_rl_repo"):
    if os.path.isdir(_p) and _p not in sys.path:
        sys.path.insert(0, _p)

import numpy as np
import ml_dtypes

import concourse.bass as bass
import concourse.bacc as bacc
import concourse.mybir as mybir
import concourse.tile as tile
from concourse.bass_utils import run_bass_kernel_spmd
from concourse.masks import make_identity

P = 128
N = 50000
E = 1500000
D = 128
L = 3
NCORES = 8
NS = N // NCORES          # 6250 nodes per core
W = 128                   # dst window (psum free dim)
NW = (NS + W - 1) // W    # 49 windows; last has 106 nodes
SPLIT = 32768             # src-index split so gather indices fit in int16
CW = 4                    # windows per gather chunk
NCHUNK = (NW + CW - 1) // CW  # 13 chunks
MLPC = 512                # mlp node-chunk (psum free dim)
OHB = 8                   # one-hot tiles built per DVE instruction

f32 = mybir.dt.float32
bf16 = mybir.dt.float16  # "bf16" name kept; fp16 halves the rounding error
i16 = mybir.dt.int16

# (t, r) -> which x history the gather reads: x_{t-r}; r=k-1
PASSES = [(t, r) for t in range(L) for r in range(t, -1, -1)]


def _wrap_idx(idx):
    # [n] -> [128, n//16] int16; element i at [i%16, i//16], replicated x8.
    arr = idx.astype(np.int16).reshape(-1, 16).T
    return np.tile(arr, (8, 1))


def _prepare(edge_index, edge_attr):
    """Host-side edge partitioning. Returns (schedule, per-core meta arrays)."""
    src = np.asarray(edge_index[0], dtype=np.int64)
    dst = np.asarray(edge_index[1], dtype=np.int64)
    attr = np.asarray(edge_attr, dtype=np.int64)

    # counts[c, r, g, w]
    counts = np.zeros((NCORES, L, 2, NW), dtype=np.int64)
    per_cr = {}
    shard = dst // NS
    for c in range(NCORES):
        for r in range(L):
            sel = (attr == r) & (shard == c)
            s = src[sel]
            dl = dst[sel] - c * NS
            g = (s >= SPLIT).astype(np.int64)
            w = dl // W
            for gi in (0, 1):
                m = g == gi
                counts[c, r, gi] = np.bincount(w[m], minlength=NW)
            per_cr[(c, r)] = (s, dl, g, w)

    # shared schedule: tiles per (r, w, g) = max over cores
    ntile = np.ceil(counts.max(axis=0) / P).astype(np.int64)  # [L, 2, NW]
    ntile = np.maximum(ntile, 1)  # keep >=1 so windows always have a group
    # per-(r,g) stream tile offsets by window
    tbase = np.zeros((L, 2, NW + 1), dtype=np.int64)
    tbase[:, :, 1:] = np.cumsum(ntile, axis=2)

    # column layout of the per-relation meta tensor:
    #   idx_g0 (nt0*8) | idx_g1 (nt1*8) | dl_g0 (nt0) | dl_g1 (nt1)
    nt_tot = tbase[:, :, -1]  # [L, 2]
    meta_cols = [int(nt_tot[r, 0] * 8 + nt_tot[r, 1] * 8 + nt_tot[r, 0] + nt_tot[r, 1])
                 for r in range(L)]

    metas = {}
    for c in range(NCORES):
        for r in range(L):
            s, dl, g, w = per_cr[(c, r)]
            parts = []
            dls = []
            for gi in (0, 1):
                m = g == gi
                sw = w[m]
                order = np.argsort(sw, kind="stable")
                sw = sw[order]
                sidx = (s[m] - gi * SPLIT)[order]
                sdl = (dl[m] % W)[order]
                total = int(nt_tot[r, gi]) * P
                # position = stream offset of window + rank within window
                cnt = counts[c, r, gi]
                within = np.arange(sw.size) - np.repeat(
                    np.concatenate([[0], np.cumsum(cnt)[:-1]]), cnt)
                pos = tbase[r, gi][sw] * P + within
                idx_pad = np.zeros(total, dtype=np.int16)
                dl_pad = np.full(total, -1, dtype=np.int16)
                idx_pad[pos] = sidx.astype(np.int16)
                dl_pad[pos] = sdl.astype(np.int16)
                parts.append(_wrap_idx(idx_pad))
                dls.append(dl_pad.reshape(-1, P).T.copy())
            metas[(c, r)] = np.concatenate(parts + dls, axis=1).astype(np.int16)

    sched = dict(ntile=ntile, tbase=tbase, nt_tot=nt_tot, meta_cols=meta_cols)
    return sched, metas


def _build(sched):
    ntile, tbase, nt_tot = sched["ntile"], sched["tbase"], sched["nt_tot"]
    meta_cols_max = max(sched["meta_cols"])

    # max tiles per (chunk, group) for gather/onehot buffer sizing
    def chunk_tiles(r, k, g):
        return int(tbase[r, g, min((k + 1) * CW, NW)] - tbase[r, g, k * CW])

    TA = max(chunk_tiles(r, k, 0) for r in range(L) for k in range(NCHUNK))
    TB = max(chunk_tiles(r, k, 1) for r in range(L) for k in range(NCHUNK))

    nc = bacc.Bacc("TRN2", num_devices=NCORES, num_swdge_queues=4)

    d_x0bf = nc.dram_tensor("x0bf", [N, D], bf16, kind="ExternalInput")
    d_xT0 = nc.dram_tensor("xT0", [P, NS], f32, kind="ExternalInput")
    d_wts = nc.dram_tensor("wts", [P, 12 * D], f32, kind="ExternalInput")
    d_bias = nc.dram_tensor("bias", [P, 12], f32, kind="ExternalInput")
    d_meta = [nc.dram_tensor(f"meta{r}", [P, sched["meta_cols"][r]], i16,
                             kind="ExternalInput") for r in range(L)]
    d_out = nc.dram_tensor("out", [NS, D], f32, kind="ExternalOutput")

    ag_in = [nc.dram_tensor(f"ag_in{t}", [NS, D], bf16, kind="Internal")
             for t in range(L - 1)]
    x_hist = [d_x0bf]
    for t in range(L - 1):
        x_hist.append(nc.dram_tensor(f"x{t + 1}bf", [N, D], bf16,
                                     kind="Internal", addr_space="Shared"))

    # weight slot order: (t, r) -> t*3+r ; self t -> 9+t
    def wslot(t, r):
        return t * 3 + r

    def sslot(t):
        return 9 + t

    # mlp node chunks
    mlp_chunks = []
    c0 = 0
    while c0 < NS:
        cw = min(MLPC, NS - c0)
        mlp_chunks.append((c0, cw))
        c0 += cw

    with tile.TileContext(nc) as tc:
        with (
            tc.tile_pool(name="persist", bufs=1) as pp,
            tc.tile_pool(name="meta", bufs=2) as pmeta,
            tc.tile_pool(name="mbuf", bufs=2) as pm,
            tc.tile_pool(name="ohbuf", bufs=2) as poh,
            tc.tile_pool(name="aggc", bufs=3) as paggc,
            tc.tile_pool(name="stage", bufs=3) as pstage,
            tc.tile_pool(name="xout", bufs=2) as pxout,
            tc.tile_pool(name="pagg", bufs=4, space="PSUM") as ps_agg,
            tc.tile_pool(name="pmlp", bufs=2, space="PSUM") as ps_mlp,
            tc.tile_pool(name="ptr", bufs=2, space="PSUM") as ps_tr,
        ):
            t_iota = pp.tile([P, P], i16, tag="iota")
            t_ident = pp.tile([P, P], f32, tag="ident")
            t_xT = pp.tile([P, NS], f32, tag="xT")
            t_acc = pp.tile([P, NS], f32, tag="acc")
            t_wall = pp.tile([P, 12 * D], f32, tag="wall")
            t_ball = pp.tile([P, 12], f32, tag="ball")

            nc.gpsimd.iota(t_iota[:], pattern=[[1, P]], base=0,
                           channel_multiplier=0)
            make_identity(nc, t_ident[:])
            nc.sync.dma_start(t_xT[:], d_xT0[:])
            nc.sync.dma_start(t_wall[:], d_wts[:])
            nc.sync.dma_start(t_ball[:], d_bias[:])

            def emit_mlp_term(t, slot, rhs_tile_of_chunk, first):
                """out_acc (+)= relu(W.T-applied rhs + b) over node chunks."""
                for (c0_, cw) in mlp_chunks:
                    pmlp = ps_mlp.tile([P, MLPC], f32, tag="pmlp")
                    nc.tensor.matmul(
                        pmlp[:, :cw],
                        lhsT=t_wall[:, slot * D:(slot + 1) * D],
                        rhs=rhs_tile_of_chunk(c0_, cw),
                        start=True, stop=True,
                    )
                    if first:
                        nc.scalar.activation(
                            t_acc[:, c0_:c0_ + cw], pmlp[:, :cw],
                            mybir.ActivationFunctionType.Relu,
                            bias=t_ball[:, slot:slot + 1])
                    else:
                        tmp = pstage.tile([P, MLPC], f32, tag="mlptmp")
                        nc.scalar.activation(
                            tmp[:, :cw], pmlp[:, :cw],
                            mybir.ActivationFunctionType.Relu,
                            bias=t_ball[:, slot:slot + 1])
                        nc.vector.tensor_add(
                            t_acc[:, c0_:c0_ + cw],
                            t_acc[:, c0_:c0_ + cw], tmp[:, :cw])

            for t in range(L):
                # self term first (only needs resident xT)
                emit_mlp_term(
                    t, sslot(t),
                    lambda c0_, cw: t_xT[:, c0_:c0_ + cw],
                    first=True)

                # relations, oldest source first (overlaps the AllGather)
                for r in range(t, -1, -1):
                    x_src = x_hist[t - r]
                    t_meta = pmeta.tile([P, meta_cols_max], i16, tag="meta")
                    nc.sync.dma_start(t_meta[:, :sched["meta_cols"][r]],
                                      d_meta[r][:])
                    icol = [0, int(nt_tot[r, 0]) * 8]
                    dcol = [int(nt_tot[r, 0] + nt_tot[r, 1]) * 8,
                            int((nt_tot[r, 0] + nt_tot[r, 1]) * 8 + nt_tot[r, 0])]

                    agg_tiles = {}  # mlp-chunk idx -> (tile, emitted windows)
                    qrr = [0]
                    for k in range(NCHUNK):
                        w_lo, w_hi = k * CW, min((k + 1) * CW, NW)
                        mb = [None, None]
                        ohb = [None, None]
                        cstart = [int(tbase[r, g, w_lo]) for g in (0, 1)]
                        for g in (0, 1):
                            ntk = chunk_tiles(r, k, g)
                            if ntk == 0:
                                continue
                            cap = TA if g == 0 else TB
                            mb[g] = pm.tile([P, cap, P], bf16,
                                            tag=f"m{g}", name=f"mb{g}")
                            nidx = ntk * P
                            src_view = (x_src[:SPLIT, :] if g == 0
                                        else x_src[SPLIT:N, :])
                            nc.gpsimd.dma_gather(
                                mb[g][:, :ntk, :], src_view,
                                t_meta[:, icol[g] + cstart[g] * 8:
                                       icol[g] + (cstart[g] + ntk) * 8],
                                nidx, nidx, D, single_packet=False,
                                queue_num=qrr[0] % 4)
                            qrr[0] += 1
                            ohb[g] = poh.tile([P, cap, P], bf16,
                                              tag=f"oh{g}", name=f"ohb{g}")
                            for j0 in range(0, ntk, OHB):
                                bs = min(OHB, ntk - j0)
                                a = t_iota[:].rearrange(
                                    "p (t w) -> p t w", t=1)
                                b = t_meta[:, dcol[g] + cstart[g] + j0:
                                           dcol[g] + cstart[g] + j0 + bs
                                           ].rearrange("p (t w) -> p t w", w=1)
                                a2, b2 = bass.broadcast_tensor_aps(a, b)
                                nc.vector.tensor_tensor(
                                    out=ohb[g][:, j0:j0 + bs, :],
                                    in0=a2, in1=b2,
                                    op=mybir.AluOpType.is_equal)

                        for w in range(w_lo, w_hi):
                            wn = min(W, NS - w * W)
                            mc = (w * W) // MLPC
                            if mc not in agg_tiles:
                                agg_tiles[mc] = paggc.tile(
                                    [P, MLPC], f32, tag="aggT", name="aggc")
                            at = agg_tiles[mc]
                            # window tile list
                            ops = []
                            for g in (0, 1):
                                lo = int(tbase[r, g, w]) - cstart[g]
                                hi = int(tbase[r, g, w + 1]) - cstart[g]
                                for j in range(lo, hi):
                                    ops.append((mb[g], ohb[g], j))
                            pagg = ps_agg.tile([P, W], f32, tag="pagg")
                            for oi, (mbt, oht, j) in enumerate(ops):
                                nc.tensor.matmul(
                                    pagg[:],
                                    lhsT=mbt[:, j, :],
                                    rhs=oht[:, j, :],
                                    start=(oi == 0),
                                    stop=(oi == len(ops) - 1))
                            off = (w * W) % MLPC
                            nc.scalar.copy(at[:, off:off + wn],
                                           pagg[:, :wn])
                            # mlp chunk complete?
                            chunk_end = (w * W + wn)
                            if chunk_end % MLPC == 0 or chunk_end == NS:
                                c0_ = mc * MLPC
                                cw = chunk_end - c0_
                                at_ref = agg_tiles.pop(mc)
                                pmlp = ps_mlp.tile([P, MLPC], f32, tag="pmlp")
                                slot = wslot(t, r)
                                nc.tensor.matmul(
                                    pmlp[:, :cw],
                                    lhsT=t_wall[:, slot * D:(slot + 1) * D],
                                    rhs=at_ref[:, :cw],
                                    start=True, stop=True)
                                tmp = pstage.tile([P, MLPC], f32, tag="mlptmp")
                                nc.scalar.activation(
                                    tmp[:, :cw], pmlp[:, :cw],
                                    mybir.ActivationFunctionType.Relu,
                                    bias=t_ball[:, slot:slot + 1])
                                nc.vector.tensor_add(
                                    t_acc[:, c0_:c0_ + cw],
                                    t_acc[:, c0_:c0_ + cw], tmp[:, :cw])

                # finalize layer: x = x + relu(acc)
                for (c0_, cw) in mlp_chunks:
                    tmp = pstage.tile([P, MLPC], f32, tag="mlptmp")
                    nc.scalar.activation(
                        tmp[:, :cw], t_acc[:, c0_:c0_ + cw],
                        mybir.ActivationFunctionType.Relu)
                    nc.vector.tensor_add(
                        t_xT[:, c0_:c0_ + cw],
                        t_xT[:, c0_:c0_ + cw], tmp[:, :cw])

                # write node-major copy (bf16 for t<2 via AllGather; f32 out at t=2)
                WB = 8  # windows per writeback batch
                for b0 in range(0, NW, WB):
                    bw = min(WB, NW - b0)
                    full_rows = min(NS, (b0 + bw) * W) - b0 * W
                    if t < L - 1:
                        st = pxout.tile([P, WB, D], bf16, tag="xbf")
                    else:
                        st = pxout.tile([P, WB, D], f32, tag="xf32")
                    for bi in range(bw):
                        w = b0 + bi
                        wn = min(W, NS - w * W)
                        ptr = ps_tr.tile([P, P], f32, tag="ptr")
                        nc.tensor.transpose(
                            out=ptr[:wn, :],
                            in_=t_xT[:, w * W:w * W + wn],
                            identity=t_ident[:])
                        nc.vector.tensor_copy(st[:wn, bi, :], ptr[:wn, :])
                    dstt = ag_in[t] if t < L - 1 else d_out
                    if full_rows % P == 0:
                        view = dstt[b0 * W: b0 * W + full_rows, :].rearrange(
                            "(c p) d -> p c d", p=P)
                        nc.sync.dma_start(view, st[:, :bw, :])
                    else:
                        nfull = full_rows // P
                        if nfull:
                            view = dstt[b0 * W: b0 * W + nfull * P, :].rearrange(
                                "(c p) d -> p c d", p=P)
                            nc.sync.dma_start(view, st[:, :nfull, :])
                        rem = full_rows - nfull * P
                        nc.sync.dma_start(
                            dstt[b0 * W + nfull * P: b0 * W + full_rows, :],
                            st[:rem, nfull, :])

                if t < L - 1:
                    nc.gpsimd.collective_compute(
                        "AllGather",
                        mybir.AluOpType.bypass,
                        replica_groups=[list(range(NCORES))],
                        ins=[ag_in[t][:]],
                        outs=[x_hist[t + 1][:]],
                    )

    nc.compile()
    return nc


_CACHE = {}


def kernel(x, Ws_s, bs_s, Ws_k, bs_k, edge_index, edge_attr):
    x = np.asarray(x, dtype=np.float32)
    Ws_s = np.asarray(Ws_s, dtype=np.float32)
    bs_s = np.asarray(bs_s, dtype=np.float32)
    Ws_k = np.asarray(Ws_k, dtype=np.float32)
    bs_k = np.asarray(bs_k, dtype=np.float32)
    edge_index = np.asarray(edge_index)
    edge_attr = np.asarray(edge_attr)

    key = hash((edge_index.tobytes(), edge_attr.tobytes()))
    if key not in _CACHE:
        sched, metas = _prepare(edge_index, edge_attr)
        nc = _build(sched)
        _CACHE[key] = (sched, metas, nc)
    sched, metas, nc = _CACHE[key]

    # pack weights: [128, 12*128]: slots t*3+r -> Ws_k[t, r]; 9+t -> Ws_s[t]
    wall = np.zeros((P, 12 * D), np.float32)
    ball = np.zeros((P, 12), np.float32)
    for t in range(L):
        for r in range(L):
            wall[:, (t * 3 + r) * D:(t * 3 + r + 1) * D] = Ws_k[t, r]
            ball[:, t * 3 + r] = bs_k[t, r]
        wall[:, (9 + t) * D:(10 + t) * D] = Ws_s[t]
        ball[:, 9 + t] = bs_s[t]

    x0bf = x.astype(np.float16)
    in_maps = []
    for c in range(NCORES):
        m = {
            "x0bf": x0bf,
            "xT0": np.ascontiguousarray(x[c * NS:(c + 1) * NS].T),
            "wts": wall,
            "bias": ball,
        }
        for r in range(L):
            m[f"meta{r}"] = metas[(c, r)]
        in_maps.append(m)

    trace = bool(int(os.environ.get("KERNEL_TRACE", "0")))
    res = run_bass_kernel_spmd(nc, in_maps, core_ids=list(range(NCORES)),
                               trace=trace)
    if trace and res.exec_time_ns is not None:
        print(f"HW exec time: {res.exec_time_ns} ns")
        kernel.last_exec_time_ns = res.exec_time_ns
        if res.instructions_and_trace is not None:
            print("trace:", res.instructions_and_trace[1])

    out = np.concatenate([res.results[c]["out"] for c in range(NCORES)], axis=0)
    return out
